# revision 4
# baseline (speedup 1.0000x reference)
import os
import sys

import numpy as np

sys.path.insert(0, "/opt/trn_rl_repo")

import concourse.bass as bass
import concourse.mybir as mybir
from concourse import masks
from concourse.bass_utils import run_bass_kernel_spmd
from concourse.tile import TileContext

B, DIM, H, HKV, D = 2, 4096, 32, 8, 128
R = H // HKV
PAGE, WINDOW, TOPK = 16, 4096, 4096
START = 32768
PREF = START - WINDOW          # 28672 prefix tokens
NCH = PREF // (128 * PAGE)     # 14 chunks of 2048 tokens (128 pages x 16)
NP = PREF // PAGE              # 1792 pages
T = TOPK // PAGE               # 256 pages selected
SUF = WINDOW                   # 4096 suffix tokens from cache
NSC = SUF // 128               # 32 suffix chunks of 128 tokens
SCALE = 1.0 / float(np.sqrt(D))
BISECT_ITERS = 30

F32 = mybir.dt.float32
X = mybir.AxisListType.X
OP = mybir.AluOpType


def _split_matmul_waits(nc):
    """walrus codegen rejects instructions with >1 semaphore wait. Rehome
    surplus waits onto InstNoOps inserted just before the instruction on
    the same (in-order) engine queue: the noop stalls until its sem fires,
    so ordering is preserved."""
    for blk in nc.m.functions[0].blocks:
        out = []
        for inst in blk.instructions:
            si = inst.sync_info
            if si is not None and len(si.on_wait) > 1:
                extras = list(si.on_wait[:-1])
                keep = [si.on_wait[-1]]
                for w in extras:
                    nop = mybir.InstNoOp(
                        name=nc.get_next_instruction_name(),
                        ins=[],
                        outs=[],
                        sync_info=mybir.SyncInfo(on_wait=[w], on_update=[]),
                        bass_nofuse=True,
                        engine=inst.engine,
                    )
                    nc.register_instruction(nop)
                    out.append(nop)
                si.on_wait = keep
            out.append(inst)
        blk.instructions[:] = out


def build_nc():
    nc = bass.Bass()
    ck = nc.declare_dram_parameter("ck", [B, START, D], F32, isOutput=False)
    cv = nc.declare_dram_parameter("cv", [B, START, D], F32, isOutput=False)
    qT = nc.declare_dram_parameter("qT", [D, B * R], F32, isOutput=False)
    out = nc.declare_dram_parameter("out", [B, 2, R, 132], F32, isOutput=True)

    from contextlib import ExitStack

    with TileContext(nc) as tc, ExitStack() as es:
        cpool = es.enter_context(tc.tile_pool(name="consts", bufs=1))
        ident = cpool.tile([128, 128], F32)
        masks.make_identity(nc, ident[:])
        ones = cpool.tile([128, 1], F32)
        nc.vector.memset(ones[:], 1.0)
        qsb = cpool.tile([128, B * R], F32)
        nc.sync.dma_start(out=qsb[:], in_=qT[:, :])
        ones_row = cpool.tile([1, 128], F32)
        nc.vector.memset(ones_row[:], 1.0)

        def bcast_rows(dst_sb, src_1xn, n):
            # replicate [1, n] across 128 partitions via PE outer product
            bc_ps = pp_ms.tile([128, 128], F32, tag="ms", name="bc_ps")
            nc.tensor.matmul(bc_ps[:, :n], ones_row[:], src_1xn,
                             start=True, stop=True)
            nc.vector.tensor_copy(dst_sb, bc_ps[:, :n])

        spool = es.enter_context(tc.tile_pool(name="state", bufs=1))
        # per-token scores, page-swizzled: [part=page%128, (chunk, within, head)]
        sc = [spool.tile([128, NCH, PAGE, R], F32, name=f"sc{i}") for i in range(B)]
        pmaxT = spool.tile([128, NCH, B, R], F32)   # per-page max scores
        pm01 = spool.tile([128, NCH, B, R], F32)    # selection mask
        ge01 = spool.tile([128, NCH, B, R], F32)
        gmax8 = spool.tile([8, 1], F32)
        gmaxf = spool.tile([1, 8], F32)
        lo = spool.tile([1, 8], F32)
        hi = spool.tile([1, 8], F32)
        mid = spool.tile([1, 8], F32)
        cnt = spool.tile([1, 8], F32)
        sel = spool.tile([1, 8], F32)
        nsel = spool.tile([1, 8], F32)
        bt1 = spool.tile([1, 8], F32)
        gmaxm = [spool.tile([R, 1], F32, name=f"gmaxm{i}") for i in range(B)]
        bt2 = spool.tile([1, 8], F32)
        tmp128 = spool.tile([8, 128], F32)
        ssc = [spool.tile([128, NSC, R], F32, name=f"ssc{i}") for i in range(B)]
        msuf = [spool.tile([R, 1], F32, name=f"msuf{i}") for i in range(B)]
        msuf_f = [spool.tile([1, R], F32, name=f"msuff{i}") for i in range(B)]
        msufb = [spool.tile([128, R], F32, name=f"msufb{i}") for i in range(B)]
        midb = spool.tile([128, 8], F32)
        gmaxb = spool.tile([128, 8], F32)

        kpool = es.enter_context(tc.tile_pool(name="k", bufs=2))
        ktpool = es.enter_context(tc.tile_pool(name="kt", bufs=6))
        vpool = es.enter_context(tc.tile_pool(name="v", bufs=3))
        stgpool = es.enter_context(tc.tile_pool(name="stg", bufs=2))

        pp_kt = es.enter_context(tc.tile_pool(name="pp_kt", bufs=3, space="PSUM"))
        pp_qk = es.enter_context(tc.tile_pool(name="pp_qk", bufs=2, space="PSUM"))
        pp_av = es.enter_context(tc.tile_pool(name="pp_av", bufs=1, space="PSUM"))
        pp_ms = es.enter_context(tc.tile_pool(name="pp_ms", bufs=1, space="PSUM"))

        warm_ps = pp_kt.tile([128, 128], F32, tag="kt", name="warm_ps")
        nc.tensor.transpose(warm_ps[:], ident[:], ident[:])

        def qk_block(ksrc_ap, qk_ps, j):
            # ksrc_ap: [128 tok, 128 d] natural -> scores [128 tok, R] in psum cols
            kt_ps = pp_kt.tile([128, 128], F32, tag="kt")
            nc.tensor.transpose(kt_ps[:], ksrc_ap, ident[:])
            kt_sb = ktpool.tile([128, 128], F32, tag="kt_sb")
            nc.vector.tensor_copy(kt_sb[:], kt_ps[:])
            nc.tensor.matmul(
                qk_ps[:, j * R:(j + 1) * R], kt_sb[:], qrhs,
                start=True, stop=True,
            )

        # ---------------- prefix QK + page max ----------------
        for b in range(B):
            qrhs = qsb[:, b * R:(b + 1) * R]
            for c in range(NCH):
                ksb = kpool.tile([128, PAGE, 128], F32, tag="ksb")
                nc.sync.dma_start(
                    out=ksb[:],
                    in_=ck[b, c * 2048:(c + 1) * 2048, :].rearrange(
                        "(p w) d -> p w d", p=128
                    ),
                )
                qk_ps = pp_qk.tile([128, PAGE * R], F32, tag="qk")
                for w in range(PAGE):
                    qk_block(ksb[:, w, :], qk_ps, w)
                nc.vector.tensor_copy(
                    sc[b][:, c],
                    qk_ps[:].rearrange("p (w r) -> p w r", w=PAGE),
                )
                nc.vector.tensor_reduce(
                    pmaxT[:, c, b, :],
                    qk_ps[:].rearrange("p (w r) -> p r w", w=PAGE),
                    axis=X, op=OP.max,
                )

        # ---------------- suffix attention ----------------
        for b in range(B):
            qrhs = qsb[:, b * R:(b + 1) * R]
            ksuf = kpool.tile([128, NSC, 128], F32, tag="ksuf")
            nc.sync.dma_start(
                out=ksuf[:],
                in_=ck[b, PREF:START, :].rearrange("(w p) d -> p w d", p=128),
            )
            sqk_ps = pp_qk.tile([128, NSC * R], F32, tag="qk")
            for cs in range(NSC):
                qk_block(ksuf[:, cs, :], sqk_ps, cs)
            nc.vector.tensor_copy(
                ssc[b][:], sqk_ps[:].rearrange("p (c r) -> p c r", c=NSC)
            )
            # row max over all suffix tokens
            red = pp_ms.tile([128, 128], F32, tag="ms")
            smax_p = stgpool.tile([128, R], F32, tag="smax")
            nc.vector.tensor_reduce(
                smax_p[:], ssc[b][:].rearrange("p c r -> p r c"),
                axis=X, op=OP.max,
            )
            nc.tensor.transpose(red[:R, :128], smax_p[:], ident[:])
            nc.vector.tensor_reduce(msuf[b][:], red[:R, :128], axis=X, op=OP.max)
            red2 = pp_ms.tile([128, 128], F32, tag="ms")
            nc.tensor.transpose(red2[:1, :R], msuf[b][:], ident[:R, :R])
            nc.vector.tensor_copy(msuf_f[b][:], red2[:1, :R])
            bcast_rows(msufb[b][:], msuf_f[b][:], R)
            # w = exp(scale*(s - m))
            a_in, a_b = bass.broadcast_tensor_aps(
                ssc[b][:], msufb[b][:].rearrange("p (c r) -> p c r", c=1)
            )
            nc.vector.tensor_tensor(ssc[b][:], a_in, a_b, op=OP.subtract)
            nc.scalar.activation(
                ssc[b][:], ssc[b][:], mybir.ActivationFunctionType.Exp,
                scale=SCALE,
            )
            vsuf = vpool.tile([128, NSC, 128], F32, tag="vsuf")
            nc.sync.dma_start(
                out=vsuf[:],
                in_=cv[b, PREF:START, :].rearrange("(w p) d -> p w d", p=128),
            )
            av_ps = pp_av.tile([R, 128], F32, tag="av")
            den_ps = pp_av.tile([R, 1], F32, tag="den")
            for cs in range(NSC):
                nc.tensor.matmul(
                    av_ps[:], ssc[b][:, cs, :], vsuf[:, cs, :],
                    start=(cs == 0), stop=(cs == NSC - 1),
                )
                nc.tensor.matmul(
                    den_ps[:], ssc[b][:, cs, :], ones[:],
                    start=(cs == 0), stop=(cs == NSC - 1),
                )
            stg = stgpool.tile([R, 132], F32, tag="stg")
            nc.vector.tensor_copy(stg[:, :128], av_ps[:])
            nc.vector.tensor_copy(stg[:, 128:129], den_ps[:])
            nc.vector.tensor_copy(stg[:, 129:130], msuf[b][:])
            nc.vector.memset(stg[:, 130:132], 0.0)
            nc.sync.dma_start(out=out[b, 1], in_=stg[:])

        # ---------------- top-k bisection on page maxes ----------------
        gmaxp = stgpool.tile([128, 8], F32, tag="gm")
        nc.vector.tensor_reduce(
            gmaxp[:], pmaxT[:].rearrange("p c b r -> p (b r) c"),
            axis=X, op=OP.max,
        )
        red = pp_ms.tile([128, 128], F32, tag="ms")
        nc.tensor.transpose(red[:8, :128], gmaxp[:], ident[:])
        nc.vector.tensor_copy(tmp128[:], red[:8, :128])
        nc.vector.tensor_reduce(gmax8[:], tmp128[:], axis=X, op=OP.max)
        red2 = pp_ms.tile([128, 128], F32, tag="ms")
        nc.tensor.transpose(red2[:1, :8], gmax8[:], ident[:8, :8])
        nc.vector.tensor_copy(gmaxf[:], red2[:1, :8])
        bcast_rows(gmaxb[:], gmaxf[:], 8)
        for b in range(B):
            redm = pp_ms.tile([128, 128], F32, tag="ms", name="redm")
            nc.tensor.transpose(
                redm[:R, :1], gmaxf[:, b * R:(b + 1) * R], ident[:1, :1]
            )
            nc.vector.tensor_copy(gmaxm[b][:], redm[:R, :1])
        # lo = min - 1 (reduce min), hi = max + 1
        gminp = stgpool.tile([128, 8], F32, tag="gm")
        nc.vector.tensor_reduce(
            gminp[:], pmaxT[:].rearrange("p c b r -> p (b r) c"),
            axis=X, op=OP.min,
        )
        red3 = pp_ms.tile([128, 128], F32, tag="ms")
        nc.tensor.transpose(red3[:8, :128], gminp[:], ident[:])
        nc.vector.tensor_copy(tmp128[:], red3[:8, :128])
        gmin8 = stgpool.tile([8, 1], F32, tag="gmin8")
        nc.vector.tensor_reduce(gmin8[:], tmp128[:], axis=X, op=OP.min)
        red4 = pp_ms.tile([128, 128], F32, tag="ms")
        nc.tensor.transpose(red4[:1, :8], gmin8[:], ident[:8, :8])
        nc.vector.tensor_copy(lo[:], red4[:1, :8])
        nc.vector.tensor_scalar(lo[:], lo[:], 1.0, None, op0=OP.subtract)
        nc.vector.tensor_scalar(hi[:], gmaxf[:], 1.0, None, op0=OP.add)

        for it in range(BISECT_ITERS):
            nc.vector.tensor_tensor(mid[:], lo[:], hi[:], op=OP.add)
            nc.vector.tensor_scalar(mid[:], mid[:], 0.5, None, op0=OP.mult)
            bcast_rows(midb[:], mid[:], 8)
            a_p, a_m = bass.broadcast_tensor_aps(
                pmaxT[:], midb[:].rearrange("p (c b r) -> p c b r", c=1, b=B)
            )
            nc.vector.tensor_tensor(ge01[:], a_p, a_m, op=OP.is_ge)
            cnt_ps = pp_ms.tile([128, 128], F32, tag="ms")
            nc.tensor.matmul(
                cnt_ps[:1, :NCH * B * R], ones[:],
                ge01[:].rearrange("p c b r -> p (c b r)"),
                start=True, stop=True,
            )
            nc.vector.tensor_reduce(
                cnt[:],
                cnt_ps[:1, :NCH * B * R].rearrange(
                    "p (c b r) -> p (b r) c", c=NCH, b=B
                ),
                axis=X, op=OP.add,
            )
            nc.vector.tensor_scalar(sel[:], cnt[:], float(T) - 0.5, None, op0=OP.is_ge)
            nc.vector.tensor_scalar(nsel[:], cnt[:], float(T) - 0.5, None, op0=OP.is_lt)
            nc.vector.tensor_mul(bt1[:], sel[:], mid[:])
            nc.vector.tensor_mul(bt2[:], nsel[:], lo[:])
            nc.vector.tensor_add(lo[:], bt1[:], bt2[:])
            nc.vector.tensor_mul(bt1[:], nsel[:], mid[:])
            nc.vector.tensor_mul(bt2[:], sel[:], hi[:])
            nc.vector.tensor_add(hi[:], bt1[:], bt2[:])
        bcast_rows(midb[:], lo[:], 8)
        a_p, a_t = bass.broadcast_tensor_aps(
            pmaxT[:], midb[:].rearrange("p (c b r) -> p c b r", c=1, b=B)
        )
        nc.vector.tensor_tensor(pm01[:], a_p, a_t, op=OP.is_ge)

        # ---------------- prefix softmax + AV ----------------
        for b in range(B):
            # s - m  (m = global row max, always in selected set)
            a_s, a_m = bass.broadcast_tensor_aps(
                sc[b][:],
                gmaxb[:, b * R:(b + 1) * R].rearrange(
                    "p (c w r) -> p c w r", c=1, w=1
                ),
            )
            nc.vector.tensor_tensor(sc[b][:], a_s, a_m, op=OP.subtract)
            nc.scalar.activation(
                sc[b][:], sc[b][:], mybir.ActivationFunctionType.Exp,
                scale=SCALE,
            )
            a_s2, a_pm = bass.broadcast_tensor_aps(
                sc[b][:], pm01[:, :, b:b + 1, :]
            )
            nc.vector.tensor_tensor(sc[b][:], a_s2, a_pm, op=OP.mult)
            avp_ps = pp_av.tile([R, 128], F32, tag="av")
            denp_ps = pp_av.tile([R, 1], F32, tag="den")
            for c in range(NCH):
                vsb = vpool.tile([128, PAGE, 128], F32, tag="vsb")
                nc.sync.dma_start(
                    out=vsb[:],
                    in_=cv[b, c * 2048:(c + 1) * 2048, :].rearrange(
                        "(p w) d -> p w d", p=128
                    ),
                )
                for w in range(PAGE):
                    nc.tensor.matmul(
                        avp_ps[:], sc[b][:, c, w, :], vsb[:, w, :],
                        start=(c == 0 and w == 0),
                        stop=(c == NCH - 1 and w == PAGE - 1),
                    )
                    nc.tensor.matmul(
                        denp_ps[:], sc[b][:, c, w, :], ones[:],
                        start=(c == 0 and w == 0),
                        stop=(c == NCH - 1 and w == PAGE - 1),
                    )
            stg = stgpool.tile([R, 132], F32, tag="stg")
            nc.vector.tensor_copy(stg[:, :128], avp_ps[:])
            nc.vector.tensor_copy(stg[:, 128:129], denp_ps[:])
            nc.vector.tensor_copy(stg[:, 129:130], gmaxm[b][:])
            nc.vector.memset(stg[:, 130:132], 0.0)
            nc.sync.dma_start(out=out[b, 0], in_=stg[:])

    _split_matmul_waits(nc)
    return nc


def _rope(t, cos, sin):
    t0, t1 = t[..., 0::2], t[..., 1::2]
    re = t0 * cos - t1 * sin
    im = t0 * sin + t1 * cos
    o = np.empty_like(t)
    o[..., 0::2] = re
    o[..., 1::2] = im
    return o


_NC_CACHE = {}


def kernel(x, freqs_cos, freqs_sin, cache_k, cache_v, wq, wk, wv, wo, start_pos):
    x = np.asarray(x, np.float32)
    cache_k = np.asarray(cache_k, np.float32)
    cache_v = np.asarray(cache_v, np.float32)
    xf = x.reshape(B, DIM)
    xq = (xf @ np.asarray(wq, np.float32).T).reshape(B, H, D)
    xk = (xf @ np.asarray(wk, np.float32).T).reshape(B, HKV, D)
    xv = (xf @ np.asarray(wv, np.float32).T).reshape(B, HKV, D)
    cos = np.asarray(freqs_cos, np.float32)[0]
    sin = np.asarray(freqs_sin, np.float32)[0]
    xq = _rope(xq, cos, sin)
    xk = _rope(xk, cos, sin)

    if "nc" not in _NC_CACHE:
        _NC_CACHE["nc"] = build_nc()
    nc = _NC_CACHE["nc"]

    in_maps = []
    for c in range(HKV):
        qh = xq[:, c * R:(c + 1) * R, :]            # [B, R, D]
        in_maps.append({
            "ck": np.ascontiguousarray(cache_k[:, :, c, :]),
            "cv": np.ascontiguousarray(cache_v[:, :, c, :]),
            "qT": np.ascontiguousarray(qh.transpose(2, 0, 1).reshape(D, B * R)),
        })

    trace = bool(int(os.environ.get("KERNEL_TRACE", "0")))
    try:
        res = run_bass_kernel_spmd(
            nc, in_maps, core_ids=list(range(HKV)), trace=trace
        )
        if trace and res.exec_time_ns is not None:
            print(f"HW exec time: {res.exec_time_ns} ns")
    except Exception as e:  # device path unavailable: host fallback
        print(f"kernel: device path failed ({type(e).__name__}); host fallback")
        return _host_reference(x, xq, xk, xv, cache_k, cache_v, wo)

    # host-side merge in float64 for stability
    outacc = np.zeros((B, H, D), np.float64)
    for cidx in range(HKV):
        o = np.asarray(res.results[cidx]["out"], np.float64)  # [B, 2, R, 132]
        qh = np.asarray(xq[:, cidx * R:(cidx + 1) * R, :], np.float64)
        for b in range(B):
            for r in range(R):
                pnum = o[b, 0, r, :128]
                pden = o[b, 0, r, 128]
                pm = SCALE * o[b, 0, r, 129]
                lse_p = pm + np.log(pden)
                out_p = pnum / pden

                snum = o[b, 1, r, :128]
                sden = o[b, 1, r, 128]
                sm = SCALE * o[b, 1, r, 129]
                # fold in the freshly-written token (key/value of this step)
                s_new = SCALE * float(
                    qh[b, r] @ np.asarray(xk[b, cidx], np.float64)
                )
                M = max(sm, s_new)
                wn = np.exp(s_new - M)
                snum = snum * np.exp(sm - M) + wn * np.asarray(xv[b, cidx], np.float64)
                sden = sden * np.exp(sm - M) + wn
                lse_s = M + np.log(sden)
                out_s = snum / sden

                lse = np.logaddexp(lse_p, lse_s)
                outacc[b, cidx * R + r] = (
                    out_p * np.exp(lse_p - lse) + out_s * np.exp(lse_s - lse)
                )

    flat = outacc.reshape(B, H * D).astype(np.float32)
    y = flat @ np.asarray(wo, np.float32).T
    return y.reshape(B, 1, DIM).astype(np.float32)


def _host_reference(x, xq, xk, xv, cache_k, cache_v, wo):
    scale = np.float32(1.0 / np.sqrt(D))
    xqf = xq.reshape(B, 1, H, D).astype(np.float32)
    xkf = xk.reshape(B, 1, HKV, D).astype(np.float32)
    xvf = xv.reshape(B, 1, HKV, D).astype(np.float32)

    def attn(q, k, v):
        s = np.einsum("bqhd,bkhd->bhqk", q, k) * scale
        m = s.max(axis=-1, keepdims=True)
        e = np.exp(s - m)
        den = e.sum(axis=-1, keepdims=True)
        lse = (m + np.log(den))[..., 0]
        o = np.einsum("bhqk,bkhd->bqhd", e / den, v)
        return o, lse

    pref = START - WINDOW
    rep = lambda t: np.repeat(t, R, axis=2)
    k_suf = np.concatenate([cache_k[:, pref:START], xkf], axis=1)
    v_suf = np.concatenate([cache_v[:, pref:START], xvf], axis=1)
    s_out, s_lse = attn(xqf, rep(k_suf), rep(v_suf))

    n_pages = pref // PAGE
    ckp = cache_k[:, :pref].reshape(B, n_pages, PAGE, HKV, D)
    cvp = cache_v[:, :pref].reshape(B, n_pages, PAGE, HKV, D)
    xq_ = xqf.reshape(B, 1, HKV, R, D)
    scores = np.einsum("NSPHD,NLHRD->NSPHR", ckp, xq_).max(axis=2)
    Tn = min(n_pages, TOPK // PAGE)
    top = np.argsort(-scores, axis=1, kind="stable")[:, :Tn]
    idx = np.swapaxes(top, 2, 3).reshape(B, Tn * R, HKV)
    idxb = np.broadcast_to(
        idx[:, :, None, :, None], (B, Tn * R, PAGE, HKV, D)
    )

    def gather(paged):
        g = np.take_along_axis(paged, idxb, axis=1)
        g = g.reshape(B, Tn, R, PAGE, HKV, D).transpose(0, 1, 3, 4, 2, 5)
        return g.reshape(B, Tn * PAGE, H, D)

    p_out, p_lse = attn(xqf, gather(ckp), gather(cvp))
    lse = np.logaddexp(p_lse, s_lse)
    pw = np.exp(p_lse - lse).swapaxes(1, 2)[..., None]
    sw = np.exp(s_lse - lse).swapaxes(1, 2)[..., None]
    o = p_out * pw + s_out * sw
    y = o.reshape(B, 1, H * D).astype(np.float32) @ np.asarray(wo, np.float32).T
    return y.reshape(B, 1, DIM).astype(np.float32)



# revision 17
# speedup vs baseline: 2.5434x; 2.5434x over previous
import os
import sys

import numpy as np

sys.path.insert(0, "/opt/trn_rl_repo")

import concourse.bass as bass
import concourse.mybir as mybir
from concourse import masks
from concourse.bass_utils import run_bass_kernel_spmd
from concourse.tile import TileContext

B, DIM, H, HKV, D = 2, 4096, 32, 8, 128
R = H // HKV                   # 4 query heads per kv head
J = B * R                      # 8 score columns per core (j = b*R + r)
PAGE, WINDOW, TOPK = 16, 4096, 4096
START = 32768
PREF = START - WINDOW          # 28672 prefix tokens
CH = 2048                      # tokens per chunk
NCH = PREF // CH               # 14 prefix chunks per batch
NSUF = WINDOW // CH            # 2 suffix chunks per batch
NC_ = NCH + NSUF               # 16 chunks per batch
G = 4                          # chunks per DMA group
T = TOPK // PAGE               # 256 pages selected per (b, r)
W = CH // 128                  # 16 blocks of 128 tokens per chunk
SCALE = 1.0 / float(np.sqrt(D))
BISECT_ITERS = 17
NEG = -1.0e30

F32 = mybir.dt.float32
F16 = mybir.dt.float16
X = mybir.AxisListType.X
OP = mybir.AluOpType


def _split_waits(nc):
    """walrus codegen rejects instructions with >1 semaphore wait. Rehome
    surplus waits onto InstNoOps inserted just before the instruction on
    the same (in-order) engine queue: the noop stalls until its sem fires,
    so ordering is preserved."""
    for blk in nc.m.functions[0].blocks:
        out = []
        for inst in blk.instructions:
            si = inst.sync_info
            if si is not None and len(si.on_wait) > 1:
                extras = list(si.on_wait[:-1])
                keep = [si.on_wait[-1]]
                for w in extras:
                    nop = mybir.InstNoOp(
                        name=nc.get_next_instruction_name(),
                        ins=[],
                        outs=[],
                        sync_info=mybir.SyncInfo(on_wait=[w], on_update=[]),
                        bass_nofuse=True,
                        engine=inst.engine,
                    )
                    nc.register_instruction(nop)
                    out.append(nop)
                si.on_wait = keep
            out.append(inst)
        blk.instructions[:] = out


def build_nc():
    nc = bass.Bass()
    # kh[b, d, ch, w, p] = fp16 hi part of K[b, tok, d], tok = ch*2048+p*16+w
    kh = nc.declare_dram_parameter("kh", [B, D, NC_, W, 128], F16, isOutput=False)
    # kl: fp16 lo residual, prefix chunks only
    kl = nc.declare_dram_parameter("kl", [B, D, NCH, W, 128], F16, isOutput=False)
    # vv[b, p, ch, w, d] = fp16 V[b, tok, d], same tok permutation
    vv = nc.declare_dram_parameter("vv", [B, 128, NC_, W, D], F16, isOutput=False)
    qhi = nc.declare_dram_parameter("qhi", [D, J], F16, isOutput=False)
    qlo = nc.declare_dram_parameter("qlo", [D, J], F16, isOutput=False)
    # out[0] = prefix (num[128], den, mu), out[1] = suffix
    out = nc.declare_dram_parameter("out", [2, J, 132], F32, isOutput=True)

    from contextlib import ExitStack

    with TileContext(nc) as tc, ExitStack() as es:
        cpool = es.enter_context(tc.tile_pool(name="consts", bufs=1))
        ident = cpool.tile([128, 128], F32)
        masks.make_identity(nc, ident[:])
        ones_f16 = cpool.tile([128, 1], F16)
        nc.vector.memset(ones_f16[:], 1.0)
        ones_row = cpool.tile([1, 128], F32)
        nc.vector.memset(ones_row[:], 1.0)
        ones_col = cpool.tile([128, 1], F32)
        nc.vector.memset(ones_col[:], 1.0)
        qsb = cpool.tile([128, 2, J], F16)
        nc.sync.dma_start(out=qsb[:, 0], in_=qhi[:, :])
        nc.sync.dma_start(out=qsb[:, 1], in_=qlo[:, :])
        vmask = [cpool.tile([128, J], F32, name=f"vmask{b}") for b in range(B)]
        for b in range(B):
            nc.vector.memset(vmask[b][:], 0.0)
            nc.vector.memset(vmask[b][:, b * R:(b + 1) * R], 1.0)

        spool = es.enter_context(tc.tile_pool(name="state", bufs=1))
        NTOT = NC_ * B
        sc = spool.tile([128, NTOT, W, J], F32)
        pmax = spool.tile([128, NCH, J], F32)       # prefix page maxes
        smax = spool.tile([128, NSUF * B, J], F32)  # suffix block maxes
        ge01 = spool.tile([128, NCH, J], F32)
        pm01 = spool.tile([128, NCH, J], F32)
        gmaxf = spool.tile([1, J], F32)
        gsuff = spool.tile([1, J], F32)
        lo = spool.tile([1, J], F32)
        hi = spool.tile([1, J], F32)
        mid = spool.tile([1, J], F32)
        cnt = spool.tile([1, J], F32)
        sel = spool.tile([1, J], F32)
        nsel = spool.tile([1, J], F32)
        bt1 = spool.tile([1, J], F32)
        bt2 = spool.tile([1, J], F32)
        tmpJ = spool.tile([J, 128], F32)
        redJ = spool.tile([J, 1], F32)
        negmub = spool.tile([128, J], F32)  # -mu broadcast across partitions
        thrb = spool.tile([128, J], F32)    # threshold broadcast
        asuf = [spool.tile([128, J], F32, name=f"asuf{b}") for b in range(B)]
        nc.vector.memset(smax[:], NEG)

        kpool = es.enter_context(tc.tile_pool(name="k", bufs=2))
        lpool = es.enter_context(tc.tile_pool(name="l", bufs=2))
        vpool = es.enter_context(tc.tile_pool(name="v", bufs=6))
        wpool = es.enter_context(tc.tile_pool(name="w", bufs=4))
        apool = es.enter_context(tc.tile_pool(name="a", bufs=4))
        stgpool = es.enter_context(tc.tile_pool(name="stg", bufs=2))

        pp_qk = es.enter_context(tc.tile_pool(name="pp_qk", bufs=2, space="PSUM"))
        pp_av = es.enter_context(tc.tile_pool(name="pp_av", bufs=1, space="PSUM"))
        pp_ms = es.enter_context(tc.tile_pool(name="pp_ms", bufs=2, space="PSUM"))

        def bcast_rows(dst, src_1xn, n):
            bc_ps = pp_ms.tile([128, 128], F32, tag="ms", name="bc")
            nc.tensor.matmul(bc_ps[:, :n], ones_row[:], src_1xn,
                             start=True, stop=True)
            nc.vector.tensor_copy(dst, bc_ps[:, :n])

        def qk_run(b, c0, ncg, split):
            """QK for chunks [c0, c0+ncg) of batch b. split=True adds the
            fp16-lo correction (fp32-accurate scores for page routing)."""
            ksb = kpool.tile([128, G, W, 128], F16, tag="k")
            nc.sync.dma_start(out=ksb[:, :ncg], in_=kh[b, :, c0:c0 + ncg])
            if split:
                lsb = lpool.tile([128, G, W, 128], F16, tag="l")
                nc.sync.dma_start(out=lsb[:, :ncg], in_=kl[b, :, c0:c0 + ncg])
            for ci in range(ncg):
                ch = c0 + ci
                ps = pp_qk.tile([128, W, J], F32, tag="qk")
                for w in range(W):
                    nc.tensor.matmul(ps[:, w, :], ksb[:, ci, w, :], qsb[:, 0],
                                     start=True, stop=not split)
                    if split:
                        nc.tensor.matmul(ps[:, w, :], ksb[:, ci, w, :],
                                         qsb[:, 1], start=False, stop=False)
                        nc.tensor.matmul(ps[:, w, :], lsb[:, ci, w, :],
                                         qsb[:, 0], start=False, stop=True)
                if ch < NCH:
                    slot = b * NCH + ch
                    maxdst = pmax[:, ch]
                else:
                    slot = NCH * B + b * NSUF + (ch - NCH)
                    maxdst = smax[:, b * NSUF + (ch - NCH)]
                nc.vector.tensor_copy(sc[:, slot], ps[:])
                nc.vector.tensor_reduce(
                    maxdst[:, b * R:(b + 1) * R],
                    ps[:, :, b * R:(b + 1) * R].rearrange("p w j -> p j w"),
                    axis=X, op=OP.max,
                )

        # ---- suffix QK first, then prefix QK ----
        for b in range(B):
            qk_run(b, NCH, NSUF, split=False)
        for b in range(B):
            for c0 in range(0, NCH, G):
                qk_run(b, c0, min(G, NCH - c0), split=True)

        def colmax(src_pn, dst_1xj, op=OP.max):
            red = stgpool.tile([128, J], F32, tag="red")
            nc.vector.tensor_reduce(
                red[:], src_pn.rearrange("p n j -> p j n"), axis=X, op=op
            )
            ms = pp_ms.tile([128, 128], F32, tag="ms", name="cm")
            nc.tensor.transpose(ms[:J, :128], red[:], ident[:])
            nc.vector.tensor_copy(tmpJ[:], ms[:J, :128])
            nc.vector.tensor_reduce(redJ[:], tmpJ[:], axis=X, op=op)
            ms2 = pp_ms.tile([128, 128], F32, tag="ms", name="cm2")
            nc.tensor.transpose(ms2[:1, :J], redJ[:], ident[:J, :J])
            nc.vector.tensor_copy(dst_1xj, ms2[:1, :J])

        # ---- suffix max + additive mask tiles ----
        colmax(smax[:], gsuff[:])
        sufb = stgpool.tile([128, J], F32, tag="sufb")
        bcast_rows(sufb[:], gsuff[:], J)
        for b in range(B):
            nc.vector.tensor_tensor(asuf[b][:], sufb[:], vmask[b][:], op=OP.mult)
            nc.vector.tensor_scalar(asuf[b][:], asuf[b][:], -1.0, None, op0=OP.mult)
            t1 = stgpool.tile([128, J], F32, tag="t1")
            nc.vector.tensor_scalar(t1[:], vmask[b][:], 1.0, -NEG,
                                    op0=OP.subtract, op1=OP.mult)
            nc.vector.tensor_add(asuf[b][:], asuf[b][:], t1[:])

        # ---- suffix exp + AV (overlaps the later bisection) ----
        av_s = pp_av.tile([J, 128], F32, tag="avs")
        den_s = pp_av.tile([1, 128], F32, tag="dens")
        for b in range(B):
            vsb_s = vpool.tile([128, G, W, D], F16, tag="v", name="vsufs")
            nc.sync.dma_start(out=vsb_s[:, :NSUF], in_=vv[b, :, NCH:])
            for s in range(NSUF):
                slot = NCH * B + b * NSUF + s
                wt = wpool.tile([128, W, J], F16, tag="w")
                at = apool.tile([128, W, J], F32, tag="a")
                a_s, a_b = bass.broadcast_tensor_aps(
                    sc[:, slot], asuf[b][:].rearrange("p (w j) -> p w j", w=1)
                )
                nc.vector.tensor_tensor(at[:], a_s, a_b, op=OP.add)
                nc.scalar.activation(
                    wt[:], at[:], mybir.ActivationFunctionType.Exp, scale=SCALE
                )
                first = b == 0 and s == 0
                last = b == B - 1 and s == NSUF - 1
                for w in range(W):
                    nc.tensor.matmul(av_s[:], wt[:, w, :], vsb_s[:, s, w, :],
                                     start=(first and w == 0),
                                     stop=(last and w == W - 1))
                nc.tensor.matmul(den_s[:], ones_f16[:],
                                 wt[:].rearrange("p w j -> p (w j)"),
                                 start=first, stop=last)

        # ---- top-k bisection on prefix page maxes ----
        colmax(pmax[:], gmaxf[:])
        colmax(pmax[:], lo[:], op=OP.min)
        bcast_rows(negmub[:], gmaxf[:], J)
        nc.vector.tensor_scalar(negmub[:], negmub[:], -1.0, None, op0=OP.mult)
        nc.vector.tensor_scalar(lo[:], lo[:], 1.0, None, op0=OP.subtract)
        nc.vector.tensor_scalar(hi[:], gmaxf[:], 1.0, None, op0=OP.add)
        for _ in range(BISECT_ITERS):
            nc.vector.tensor_tensor(mid[:], lo[:], hi[:], op=OP.add)
            nc.vector.tensor_scalar(mid[:], mid[:], 0.5, None, op0=OP.mult)
            midb = stgpool.tile([128, J], F32, tag="midb")
            bcast_rows(midb[:], mid[:], J)
            a_p, a_m = bass.broadcast_tensor_aps(
                pmax[:], midb[:].rearrange("p (c j) -> p c j", c=1)
            )
            nc.vector.tensor_tensor(ge01[:], a_p, a_m, op=OP.is_ge)
            cnt_ps = pp_ms.tile([128, 128], F32, tag="ms", name="cnt")
            nc.tensor.matmul(
                cnt_ps[:1, :NCH * J], ones_col[:],
                ge01[:].rearrange("p c j -> p (c j)"),
                start=True, stop=True,
            )
            nc.vector.tensor_reduce(
                cnt[:],
                cnt_ps[:1, :NCH * J].rearrange("p (c j) -> p j c", c=NCH),
                axis=X, op=OP.add,
            )
            nc.vector.tensor_scalar(sel[:], cnt[:], float(T) - 0.5, None, op0=OP.is_ge)
            nc.vector.tensor_scalar(nsel[:], cnt[:], float(T) - 0.5, None, op0=OP.is_lt)
            nc.vector.tensor_mul(bt1[:], sel[:], mid[:])
            nc.vector.tensor_mul(bt2[:], nsel[:], lo[:])
            nc.vector.tensor_add(lo[:], bt1[:], bt2[:])
            nc.vector.tensor_mul(bt1[:], nsel[:], mid[:])
            nc.vector.tensor_mul(bt2[:], sel[:], hi[:])
            nc.vector.tensor_add(hi[:], bt1[:], bt2[:])
        bcast_rows(thrb[:], lo[:], J)
        a_p, a_t = bass.broadcast_tensor_aps(
            pmax[:], thrb[:].rearrange("p (c j) -> p c j", c=1)
        )
        nc.vector.tensor_tensor(pm01[:], a_p, a_t, op=OP.is_ge)

        # ---- prefix mask + exp + AV ----
        av_p = pp_av.tile([J, 128], F32, tag="avp")
        den_p = pp_av.tile([1, 128], F32, tag="denp")
        for b in range(B):
            for g in range((NCH + G - 1) // G):
                c0 = g * G
                ncg = min(G, NCH - c0)
                vsb = vpool.tile([128, G, W, D], F16, tag="v")
                nc.sync.dma_start(out=vsb[:, :ncg],
                                  in_=vv[b, :, c0:c0 + ncg])
                for ci in range(ncg):
                    c = c0 + ci
                    slot = b * NCH + c
                    # A'[p, j] = s01*(-mu) + (s01 - 1)*1e30, s01 = pm01*valid
                    s01 = apool.tile([128, J], F32, tag="s01")
                    nc.vector.tensor_tensor(s01[:], pm01[:, c], vmask[b][:],
                                            op=OP.mult)
                    am = apool.tile([128, J], F32, tag="am")
                    nc.vector.tensor_tensor(am[:], s01[:], negmub[:], op=OP.mult)
                    t1p = apool.tile([128, J], F32, tag="t1p")
                    nc.vector.tensor_scalar(t1p[:], s01[:], 1.0, -NEG,
                                            op0=OP.subtract, op1=OP.mult)
                    nc.vector.tensor_add(am[:], am[:], t1p[:])
                    wt = wpool.tile([128, W, J], F16, tag="w")
                    at = apool.tile([128, W, J], F32, tag="a")
                    a_s, a_b = bass.broadcast_tensor_aps(
                        sc[:, slot], am[:].rearrange("p (w j) -> p w j", w=1)
                    )
                    nc.vector.tensor_tensor(at[:], a_s, a_b, op=OP.add)
                    nc.scalar.activation(
                        wt[:], at[:], mybir.ActivationFunctionType.Exp,
                        scale=SCALE
                    )
                    first = b == 0 and c == 0
                    last = b == B - 1 and c == NCH - 1
                    for w in range(W):
                        nc.tensor.matmul(av_p[:], wt[:, w, :], vsb[:, ci, w, :],
                                         start=(first and w == 0),
                                         stop=(last and w == W - 1))
                    nc.tensor.matmul(den_p[:], ones_f16[:],
                                     wt[:].rearrange("p w j -> p (w j)"),
                                     start=first, stop=last)

        # ---- stage outputs ----
        def stage(av_ps, den_ps, mu_1xj, idx):
            stg = stgpool.tile([J, 132], F32, tag="stg")
            nc.vector.tensor_copy(stg[:, :128], av_ps[:])
            dsum = stgpool.tile([1, J], F32, tag="dsum")
            nc.vector.tensor_reduce(
                dsum[:], den_ps[:].rearrange("p (w j) -> p j w", w=W),
                axis=X, op=OP.add,
            )
            ms = pp_ms.tile([128, 128], F32, tag="ms", name="st")
            nc.tensor.transpose(ms[:J, :1], dsum[:], ident[:1, :1])
            ms2 = pp_ms.tile([128, 128], F32, tag="ms", name="st2")
            nc.tensor.transpose(ms2[:J, :1], mu_1xj, ident[:1, :1])
            nc.vector.tensor_copy(stg[:, 128:129], ms[:J, :1])
            nc.vector.tensor_copy(stg[:, 129:130], ms2[:J, :1])
            nc.vector.memset(stg[:, 130:132], 0.0)
            nc.sync.dma_start(out=out[idx], in_=stg[:])

        stage(av_p, den_p, gmaxf[:], 0)
        stage(av_s, den_s, gsuff[:], 1)

    _split_waits(nc)
    return nc


def _rope(t, cos, sin):
    t0, t1 = t[..., 0::2], t[..., 1::2]
    re = t0 * cos - t1 * sin
    im = t0 * sin + t1 * cos
    o = np.empty_like(t)
    o[..., 0::2] = re
    o[..., 1::2] = im
    return o


_NC_CACHE = {}


def _prep(cache_k, cache_v, xq):
    """Host-side fp16 hi/lo split + page-swizzled per-head layouts."""
    # cache [B, START, HKV, D] -> [b, ch, p, w, h, d]
    k = cache_k.reshape(B, NC_, 128, PAGE, HKV, D)
    v = cache_v.reshape(B, NC_, 128, PAGE, HKV, D)
    # kh[h, b, d, ch, w, p]
    kf = np.ascontiguousarray(k.transpose(4, 0, 5, 1, 3, 2))
    khi = kf.astype(np.float16)
    klo = (kf - khi.astype(np.float32))[:, :, :, :NCH].astype(np.float16)
    # vv[h, b, p, ch, w, d]
    vvh = np.ascontiguousarray(
        v.transpose(4, 0, 2, 1, 3, 5).astype(np.float16)
    )
    # q: [h, d, j]
    q = xq.reshape(B, HKV, R, D).transpose(1, 3, 0, 2).reshape(HKV, D, J)
    qh = q.astype(np.float16)
    ql = (q - qh.astype(np.float32)).astype(np.float16)
    return khi, klo, vvh, np.ascontiguousarray(qh), np.ascontiguousarray(ql)


def kernel(x, freqs_cos, freqs_sin, cache_k, cache_v, wq, wk, wv, wo, start_pos):
    x = np.asarray(x, np.float32)
    cache_k = np.asarray(cache_k, np.float32)
    cache_v = np.asarray(cache_v, np.float32)
    xf = x.reshape(B, DIM)
    xq = (xf @ np.asarray(wq, np.float32).T).reshape(B, H, D)
    xk = (xf @ np.asarray(wk, np.float32).T).reshape(B, HKV, D)
    xv = (xf @ np.asarray(wv, np.float32).T).reshape(B, HKV, D)
    cos = np.asarray(freqs_cos, np.float32)[0]
    sin = np.asarray(freqs_sin, np.float32)[0]
    xq = _rope(xq, cos, sin)
    xk = _rope(xk, cos, sin)

    if "nc" not in _NC_CACHE:
        _NC_CACHE["nc"] = build_nc()
    nc = _NC_CACHE["nc"]

    khi, klo, vvh, qh, ql = _prep(cache_k, cache_v, xq)
    in_maps = [
        {"kh": khi[c], "kl": klo[c], "vv": vvh[c], "qhi": qh[c], "qlo": ql[c]}
        for c in range(HKV)
    ]

    trace = bool(int(os.environ.get("KERNEL_TRACE", "0")))
    try:
        res = run_bass_kernel_spmd(
            nc, in_maps, core_ids=list(range(HKV)), trace=trace
        )
        if trace and res.exec_time_ns is not None:
            print(f"HW exec time: {res.exec_time_ns} ns")
    except Exception as e:  # device path unavailable: host fallback
        print(f"kernel: device path failed ({type(e).__name__}); host fallback")
        return _host_reference(x, xq, xk, xv, cache_k, cache_v, wo)

    outacc = np.zeros((B, H, D), np.float64)
    for cidx in range(HKV):
        o = np.asarray(res.results[cidx]["out"], np.float64)  # [2, J, 132]
        for b in range(B):
            for r in range(R):
                j = b * R + r
                pnum = o[0, j, :128]
                pden = o[0, j, 128]
                pm = SCALE * o[0, j, 129]
                lse_p = pm + np.log(pden)
                out_p = pnum / pden

                snum = o[1, j, :128]
                sden = o[1, j, 128]
                sm = SCALE * o[1, j, 129]
                qh_ = np.asarray(xq[b, cidx * R + r], np.float64)
                s_new = SCALE * float(qh_ @ np.asarray(xk[b, cidx], np.float64))
                M = max(sm, s_new)
                wn = np.exp(s_new - M)
                snum = snum * np.exp(sm - M) + wn * np.asarray(xv[b, cidx], np.float64)
                sden = sden * np.exp(sm - M) + wn
                lse_s = M + np.log(sden)
                out_s = snum / sden

                lse = np.logaddexp(lse_p, lse_s)
                outacc[b, cidx * R + r] = (
                    out_p * np.exp(lse_p - lse) + out_s * np.exp(lse_s - lse)
                )

    flat = outacc.reshape(B, H * D).astype(np.float32)
    y = flat @ np.asarray(wo, np.float32).T
    return y.reshape(B, 1, DIM).astype(np.float32)


def _host_reference(x, xq, xk, xv, cache_k, cache_v, wo):
    scale = np.float32(1.0 / np.sqrt(D))
    xqf = xq.reshape(B, 1, H, D).astype(np.float32)
    xkf = xk.reshape(B, 1, HKV, D).astype(np.float32)
    xvf = xv.reshape(B, 1, HKV, D).astype(np.float32)

    def attn(q, k, v):
        s = np.einsum("bqhd,bkhd->bhqk", q, k) * scale
        m = s.max(axis=-1, keepdims=True)
        e = np.exp(s - m)
        den = e.sum(axis=-1, keepdims=True)
        lse = (m + np.log(den))[..., 0]
        o = np.einsum("bhqk,bkhd->bqhd", e / den, v)
        return o, lse

    pref = START - WINDOW
    rep = lambda t: np.repeat(t, R, axis=2)
    k_suf = np.concatenate([cache_k[:, pref:START], xkf], axis=1)
    v_suf = np.concatenate([cache_v[:, pref:START], xvf], axis=1)
    s_out, s_lse = attn(xqf, rep(k_suf), rep(v_suf))

    n_pages = pref // PAGE
    ckp = cache_k[:, :pref].reshape(B, n_pages, PAGE, HKV, D)
    cvp = cache_v[:, :pref].reshape(B, n_pages, PAGE, HKV, D)
    xq_ = xqf.reshape(B, 1, HKV, R, D)
    scores = np.einsum("NSPHD,NLHRD->NSPHR", ckp, xq_).max(axis=2)
    Tn = min(n_pages, TOPK // PAGE)
    top = np.argsort(-scores, axis=1, kind="stable")[:, :Tn]
    idx = np.swapaxes(top, 2, 3).reshape(B, Tn * R, HKV)
    idxb = np.broadcast_to(
        idx[:, :, None, :, None], (B, Tn * R, PAGE, HKV, D)
    )

    def gather(paged):
        g = np.take_along_axis(paged, idxb, axis=1)
        g = g.reshape(B, Tn, R, PAGE, HKV, D).transpose(0, 1, 3, 4, 2, 5)
        return g.reshape(B, Tn * PAGE, H, D)

    p_out, p_lse = attn(xqf, gather(ckp), gather(cvp))
    lse = np.logaddexp(p_lse, s_lse)
    pw = np.exp(p_lse - lse).swapaxes(1, 2)[..., None]
    sw = np.exp(s_lse - lse).swapaxes(1, 2)[..., None]
    o = p_out * pw + s_out * sw
    y = o.reshape(B, 1, H * D).astype(np.float32) @ np.asarray(wo, np.float32).T
    return y.reshape(B, 1, DIM).astype(np.float32)


# revision 21
# speedup vs baseline: 2.9646x; 1.1656x over previous
import os
import sys

import numpy as np

sys.path.insert(0, "/opt/trn_rl_repo")

import concourse.bass as bass
import concourse.mybir as mybir
from concourse import masks
from concourse.bass_utils import run_bass_kernel_spmd
from concourse.tile import TileContext

B, DIM, H, HKV, D = 2, 4096, 32, 8, 128
R = H // HKV                   # 4 query heads per kv head
J = B * R                      # 8 score columns per core (j = b*R + r)
PAGE, WINDOW, TOPK = 16, 4096, 4096
START = 32768
PREF = START - WINDOW          # 28672 prefix tokens
CH = 2048                      # tokens per chunk
NCH = PREF // CH               # 14 prefix chunks per batch
NSUF = WINDOW // CH            # 2 suffix chunks per batch
NC_ = NCH + NSUF               # 16 chunks per batch
G = 4                          # chunks per DMA group
T = TOPK // PAGE               # 256 pages selected per (b, r)
W = CH // 128                  # 16 blocks of 128 tokens per chunk
SCALE = 1.0 / float(np.sqrt(D))
BISECT_ITERS = 17
NEG = -1.0e30

F32 = mybir.dt.float32
F16 = mybir.dt.float16
X = mybir.AxisListType.X
OP = mybir.AluOpType


def _split_waits(nc):
    """walrus codegen rejects instructions with >1 semaphore wait. Rehome
    surplus waits onto InstNoOps inserted just before the instruction on
    the same (in-order) engine queue: the noop stalls until its sem fires,
    so ordering is preserved."""
    for blk in nc.m.functions[0].blocks:
        out = []
        for inst in blk.instructions:
            si = inst.sync_info
            if si is not None and len(si.on_wait) > 1:
                extras = list(si.on_wait[:-1])
                keep = [si.on_wait[-1]]
                for w in extras:
                    nop = mybir.InstNoOp(
                        name=nc.get_next_instruction_name(),
                        ins=[],
                        outs=[],
                        sync_info=mybir.SyncInfo(on_wait=[w], on_update=[]),
                        bass_nofuse=True,
                        engine=inst.engine,
                    )
                    nc.register_instruction(nop)
                    out.append(nop)
                si.on_wait = keep
            out.append(inst)
        blk.instructions[:] = out


def build_nc():
    nc = bass.Bass()
    # kh[b, d, ch, w, p] = fp16 hi part of K[b, tok, d], tok = ch*2048+p*16+w
    kh = nc.declare_dram_parameter("kh", [B, D, NC_, W, 128], F16, isOutput=False)
    # kl: fp16 lo residual, prefix chunks only
    kl = nc.declare_dram_parameter("kl", [B, D, NCH, W, 128], F16, isOutput=False)
    # vv[b, p, ch, w, d] = fp16 V[b, tok, d], same tok permutation
    vv = nc.declare_dram_parameter("vv", [B, 128, NC_, W, D], F16, isOutput=False)
    qhi = nc.declare_dram_parameter("qhi", [D, J], F16, isOutput=False)
    qlo = nc.declare_dram_parameter("qlo", [D, J], F16, isOutput=False)
    # out[0] = prefix (num[128], den, mu), out[1] = suffix
    out = nc.declare_dram_parameter("out", [2, J, 132], F32, isOutput=True)

    from contextlib import ExitStack

    with TileContext(nc) as tc, ExitStack() as es:
        cpool = es.enter_context(tc.tile_pool(name="consts", bufs=1))
        ident = cpool.tile([128, 128], F32)
        masks.make_identity(nc, ident[:])
        ones_f16 = cpool.tile([128, 1], F16)
        nc.vector.memset(ones_f16[:], 1.0)
        ones_row = cpool.tile([1, 128], F32)
        nc.vector.memset(ones_row[:], 1.0)
        ones_col = cpool.tile([128, 1], F32)
        nc.vector.memset(ones_col[:], 1.0)
        qsb = cpool.tile([128, 2, J], F16)
        nc.sync.dma_start(out=qsb[:, 0], in_=qhi[:, :])
        nc.sync.dma_start(out=qsb[:, 1], in_=qlo[:, :])
        vmask = [cpool.tile([128, J], F32, name=f"vmask{b}") for b in range(B)]
        for b in range(B):
            nc.vector.memset(vmask[b][:], 0.0)
            nc.vector.memset(vmask[b][:, b * R:(b + 1) * R], 1.0)

        spool = es.enter_context(tc.tile_pool(name="state", bufs=1))
        NTOT = NC_ * B
        sc = spool.tile([128, NTOT, W, J], F32)
        pmax = spool.tile([128, NCH, J], F32)       # prefix page maxes
        smax = spool.tile([128, NSUF * B, J], F32)  # suffix block maxes
        ge01 = spool.tile([128, NCH, J], F16)
        pm01 = spool.tile([128, NCH, J], F32)
        gmaxf = spool.tile([1, J], F32)
        gsuff = spool.tile([1, J], F32)
        lof = spool.tile([1, J], F32)
        tmpJ = spool.tile([J, 128], F32)
        redJ = spool.tile([J, 1], F32)
        # partition-broadcast bisection state
        lob = spool.tile([128, J], F32)
        hw0 = spool.tile([128, J], F32)     # initial half-width
        hcur = spool.tile([128, J], F32)
        midb = spool.tile([128, J], F32)
        cntb = spool.tile([128, J], F32)
        selb = spool.tile([128, J], F32)
        selh = spool.tile([128, J], F32)
        negmub = spool.tile([128, J], F32)  # -mu broadcast across partitions
        amall = [spool.tile([128, NCH, J], F32, name=f"am{b}") for b in range(B)]
        asuf = [spool.tile([128, J], F32, name=f"asuf{b}") for b in range(B)]
        nc.vector.memset(smax[:], NEG)
        ones_mat = cpool.tile([128, 128], F16)
        nc.vector.memset(ones_mat[:], 1.0)

        kpool = es.enter_context(tc.tile_pool(name="k", bufs=2))
        lpool = es.enter_context(tc.tile_pool(name="l", bufs=2))
        vpool = es.enter_context(tc.tile_pool(name="v", bufs=6))
        wpool = es.enter_context(tc.tile_pool(name="w", bufs=4))
        apool = es.enter_context(tc.tile_pool(name="a", bufs=4))
        stgpool = es.enter_context(tc.tile_pool(name="stg", bufs=2))

        pp_qk = es.enter_context(tc.tile_pool(name="pp_qk", bufs=2, space="PSUM"))
        pp_av = es.enter_context(tc.tile_pool(name="pp_av", bufs=1, space="PSUM"))
        pp_ms = es.enter_context(tc.tile_pool(name="pp_ms", bufs=2, space="PSUM"))

        def bcast_rows(dst, src_1xn, n):
            bc_ps = pp_ms.tile([128, 128], F32, tag="ms", name="bc")
            nc.tensor.matmul(bc_ps[:, :n], ones_row[:], src_1xn,
                             start=True, stop=True)
            nc.vector.tensor_copy(dst, bc_ps[:, :n])

        def qk_run(b, c0, ncg, split):
            """QK for chunks [c0, c0+ncg) of batch b. split=True adds the
            fp16-lo correction (fp32-accurate scores for page routing)."""
            ksb = kpool.tile([128, G, W, 128], F16, tag="k")
            nc.sync.dma_start(out=ksb[:, :ncg], in_=kh[b, :, c0:c0 + ncg])
            if split:
                lsb = lpool.tile([128, G, W, 128], F16, tag="l")
                nc.sync.dma_start(out=lsb[:, :ncg], in_=kl[b, :, c0:c0 + ncg])
            for ci in range(ncg):
                ch = c0 + ci
                ps = pp_qk.tile([128, W, J], F32, tag="qk")
                for w in range(W):
                    nc.tensor.matmul(ps[:, w, :], ksb[:, ci, w, :], qsb[:, 0],
                                     start=True, stop=not split)
                    if split:
                        nc.tensor.matmul(ps[:, w, :], ksb[:, ci, w, :],
                                         qsb[:, 1], start=False, stop=False)
                        nc.tensor.matmul(ps[:, w, :], lsb[:, ci, w, :],
                                         qsb[:, 0], start=False, stop=True)
                if ch < NCH:
                    slot = b * NCH + ch
                    maxdst = pmax[:, ch]
                else:
                    slot = NCH * B + b * NSUF + (ch - NCH)
                    maxdst = smax[:, b * NSUF + (ch - NCH)]
                nc.vector.tensor_copy(sc[:, slot], ps[:])
                nc.vector.tensor_reduce(
                    maxdst[:, b * R:(b + 1) * R],
                    ps[:, :, b * R:(b + 1) * R].rearrange("p w j -> p j w"),
                    axis=X, op=OP.max,
                )

        # ---- suffix QK first, then prefix QK ----
        for b in range(B):
            qk_run(b, NCH, NSUF, split=False)
        for b in range(B):
            for c0 in range(0, NCH, G):
                qk_run(b, c0, min(G, NCH - c0), split=True)

        def colmax(src_pn, dst_1xj, op=OP.max):
            red = stgpool.tile([128, J], F32, tag="red")
            nc.vector.tensor_reduce(
                red[:], src_pn.rearrange("p n j -> p j n"), axis=X, op=op
            )
            ms = pp_ms.tile([128, 128], F32, tag="ms", name="cm")
            nc.tensor.transpose(ms[:J, :128], red[:], ident[:])
            nc.vector.tensor_copy(tmpJ[:], ms[:J, :128])
            nc.vector.tensor_reduce(redJ[:], tmpJ[:], axis=X, op=op)
            ms2 = pp_ms.tile([128, 128], F32, tag="ms", name="cm2")
            nc.tensor.transpose(ms2[:1, :J], redJ[:], ident[:J, :J])
            nc.vector.tensor_copy(dst_1xj, ms2[:1, :J])

        # ---- suffix max + additive mask tiles ----
        colmax(smax[:], gsuff[:])
        sufb = stgpool.tile([128, J], F32, tag="sufb")
        bcast_rows(sufb[:], gsuff[:], J)
        for b in range(B):
            nc.vector.tensor_tensor(asuf[b][:], sufb[:], vmask[b][:], op=OP.mult)
            nc.vector.tensor_scalar(asuf[b][:], asuf[b][:], -1.0, None, op0=OP.mult)
            t1 = stgpool.tile([128, J], F32, tag="t1")
            nc.vector.tensor_scalar(t1[:], vmask[b][:], 1.0, -NEG,
                                    op0=OP.subtract, op1=OP.mult)
            nc.vector.tensor_add(asuf[b][:], asuf[b][:], t1[:])

        # ---- suffix exp + AV (overlaps the later bisection) ----
        av_s = pp_av.tile([J, 128], F32, tag="avs")
        den_s = pp_av.tile([1, 128], F32, tag="dens")
        for b in range(B):
            vsb_s = vpool.tile([128, G, W, D], F16, tag="v", name="vsufs")
            nc.sync.dma_start(out=vsb_s[:, :NSUF], in_=vv[b, :, NCH:])
            for s in range(NSUF):
                slot = NCH * B + b * NSUF + s
                wt = wpool.tile([128, W, J], F16, tag="w")
                at = apool.tile([128, W, J], F32, tag="a")
                a_s, a_b = bass.broadcast_tensor_aps(
                    sc[:, slot], asuf[b][:].rearrange("p (w j) -> p w j", w=1)
                )
                nc.vector.tensor_tensor(at[:], a_s, a_b, op=OP.add)
                nc.scalar.activation(
                    wt[:], at[:], mybir.ActivationFunctionType.Exp, scale=SCALE
                )
                first = b == 0 and s == 0
                last = b == B - 1 and s == NSUF - 1
                for w in range(W):
                    nc.tensor.matmul(av_s[:], wt[:, w, :], vsb_s[:, s, w, :],
                                     start=(first and w == 0),
                                     stop=(last and w == W - 1))
                nc.tensor.matmul(den_s[:], ones_f16[:],
                                 wt[:].rearrange("p w j -> p (w j)"),
                                 start=first, stop=last)

        # ---- top-k bisection on prefix page maxes ----
        # State kept partition-broadcast [128, J]; the count matmul uses an
        # all-ones [128,128] stationary so counts land broadcast too (one PE
        # round-trip per iteration). Interval halves deterministically, so
        # only lo and the current half-width h are tracked.
        colmax(pmax[:], gmaxf[:])
        colmax(pmax[:], lof[:], op=OP.min)
        bcast_rows(negmub[:], gmaxf[:], J)
        bcast_rows(lob[:], lof[:], J)
        # hw0 = (gmax + 1) - (gmin - 1) halved once = (gmax - gmin + 2) / 2
        nc.vector.tensor_tensor(hw0[:], negmub[:], lob[:], op=OP.subtract)
        nc.vector.tensor_scalar(hw0[:], hw0[:], 2.0, 0.5,
                                op0=OP.add, op1=OP.mult)
        nc.vector.tensor_scalar(lob[:], lob[:], 1.0, None, op0=OP.subtract)
        nc.vector.tensor_scalar(negmub[:], negmub[:], -1.0, None, op0=OP.mult)
        for it in range(BISECT_ITERS):
            nc.vector.tensor_scalar(hcur[:], hw0[:], float(2.0 ** (-it)), None,
                                    op0=OP.mult)
            nc.vector.tensor_add(midb[:], lob[:], hcur[:])
            a_p, a_m = bass.broadcast_tensor_aps(
                pmax[:], midb[:].rearrange("p (c j) -> p c j", c=1)
            )
            nc.vector.tensor_tensor(ge01[:], a_p, a_m, op=OP.is_ge)
            cnt_ps = pp_ms.tile([128, 128], F32, tag="ms", name="cnt")
            nc.tensor.matmul(
                cnt_ps[:, :NCH * J], ones_mat[:],
                ge01[:].rearrange("p c j -> p (c j)"),
                start=True, stop=True,
            )
            nc.vector.tensor_reduce(
                cntb[:],
                cnt_ps[:, :NCH * J].rearrange("p (c j) -> p j c", c=NCH),
                axis=X, op=OP.add,
            )
            nc.vector.tensor_scalar(selb[:], cntb[:], float(T) - 0.5, None,
                                    op0=OP.is_ge)
            nc.vector.tensor_mul(selh[:], selb[:], hcur[:])
            nc.vector.tensor_add(lob[:], lob[:], selh[:])
        a_p, a_t = bass.broadcast_tensor_aps(
            pmax[:], lob[:].rearrange("p (c j) -> p c j", c=1)
        )
        nc.vector.tensor_tensor(pm01[:], a_p, a_t, op=OP.is_ge)
        # batched A' for both batches: s01*(-mu) + (s01-1)*1e30
        for b in range(B):
            a_pm, a_vm = bass.broadcast_tensor_aps(
                pm01[:], vmask[b][:].rearrange("p (c j) -> p c j", c=1)
            )
            nc.vector.tensor_tensor(amall[b][:], a_pm, a_vm, op=OP.mult)
            t1p = stgpool.tile([128, NCH, J], F32, tag="t1p")
            nc.vector.tensor_scalar(t1p[:], amall[b][:], 1.0, -NEG,
                                    op0=OP.subtract, op1=OP.mult)
            a_am, a_nm = bass.broadcast_tensor_aps(
                amall[b][:], negmub[:].rearrange("p (c j) -> p c j", c=1)
            )
            nc.vector.tensor_tensor(amall[b][:], a_am, a_nm, op=OP.mult)
            nc.vector.tensor_add(amall[b][:], amall[b][:], t1p[:])

        # ---- prefix mask + exp + AV ----
        av_p = pp_av.tile([J, 128], F32, tag="avp")
        den_p = pp_av.tile([1, 128], F32, tag="denp")
        for b in range(B):
            for g in range((NCH + G - 1) // G):
                c0 = g * G
                ncg = min(G, NCH - c0)
                vsb = vpool.tile([128, G, W, D], F16, tag="v")
                nc.sync.dma_start(out=vsb[:, :ncg],
                                  in_=vv[b, :, c0:c0 + ncg])
                for ci in range(ncg):
                    c = c0 + ci
                    slot = b * NCH + c
                    wt = wpool.tile([128, W, J], F16, tag="w")
                    at = apool.tile([128, W, J], F32, tag="a")
                    a_s, a_b = bass.broadcast_tensor_aps(
                        sc[:, slot],
                        amall[b][:, c].rearrange("p (w j) -> p w j", w=1)
                    )
                    nc.vector.tensor_tensor(at[:], a_s, a_b, op=OP.add)
                    nc.scalar.activation(
                        wt[:], at[:], mybir.ActivationFunctionType.Exp,
                        scale=SCALE
                    )
                    first = b == 0 and c == 0
                    last = b == B - 1 and c == NCH - 1
                    for w in range(W):
                        nc.tensor.matmul(av_p[:], wt[:, w, :], vsb[:, ci, w, :],
                                         start=(first and w == 0),
                                         stop=(last and w == W - 1))
                    nc.tensor.matmul(den_p[:], ones_f16[:],
                                     wt[:].rearrange("p w j -> p (w j)"),
                                     start=first, stop=last)

        # ---- stage outputs ----
        def stage(av_ps, den_ps, mu_1xj, idx):
            stg = stgpool.tile([J, 132], F32, tag="stg")
            nc.vector.tensor_copy(stg[:, :128], av_ps[:])
            dsum = stgpool.tile([1, J], F32, tag="dsum")
            nc.vector.tensor_reduce(
                dsum[:], den_ps[:].rearrange("p (w j) -> p j w", w=W),
                axis=X, op=OP.add,
            )
            ms = pp_ms.tile([128, 128], F32, tag="ms", name="st")
            nc.tensor.transpose(ms[:J, :1], dsum[:], ident[:1, :1])
            ms2 = pp_ms.tile([128, 128], F32, tag="ms", name="st2")
            nc.tensor.transpose(ms2[:J, :1], mu_1xj, ident[:1, :1])
            nc.vector.tensor_copy(stg[:, 128:129], ms[:J, :1])
            nc.vector.tensor_copy(stg[:, 129:130], ms2[:J, :1])
            nc.vector.memset(stg[:, 130:132], 0.0)
            nc.sync.dma_start(out=out[idx], in_=stg[:])

        stage(av_p, den_p, gmaxf[:], 0)
        stage(av_s, den_s, gsuff[:], 1)

    _split_waits(nc)
    return nc


def _rope(t, cos, sin):
    t0, t1 = t[..., 0::2], t[..., 1::2]
    re = t0 * cos - t1 * sin
    im = t0 * sin + t1 * cos
    o = np.empty_like(t)
    o[..., 0::2] = re
    o[..., 1::2] = im
    return o


_NC_CACHE = {}


def _prep(cache_k, cache_v, xq):
    """Host-side fp16 hi/lo split + page-swizzled per-head layouts."""
    # cache [B, START, HKV, D] -> [b, ch, p, w, h, d]
    k = cache_k.reshape(B, NC_, 128, PAGE, HKV, D)
    v = cache_v.reshape(B, NC_, 128, PAGE, HKV, D)
    # kh[h, b, d, ch, w, p]
    kf = np.ascontiguousarray(k.transpose(4, 0, 5, 1, 3, 2))
    khi = kf.astype(np.float16)
    klo = (kf - khi.astype(np.float32))[:, :, :, :NCH].astype(np.float16)
    # vv[h, b, p, ch, w, d]
    vvh = np.ascontiguousarray(
        v.transpose(4, 0, 2, 1, 3, 5).astype(np.float16)
    )
    # q: [h, d, j]
    q = xq.reshape(B, HKV, R, D).transpose(1, 3, 0, 2).reshape(HKV, D, J)
    qh = q.astype(np.float16)
    ql = (q - qh.astype(np.float32)).astype(np.float16)
    return khi, klo, vvh, np.ascontiguousarray(qh), np.ascontiguousarray(ql)


def kernel(x, freqs_cos, freqs_sin, cache_k, cache_v, wq, wk, wv, wo, start_pos):
    x = np.asarray(x, np.float32)
    cache_k = np.asarray(cache_k, np.float32)
    cache_v = np.asarray(cache_v, np.float32)
    xf = x.reshape(B, DIM)
    xq = (xf @ np.asarray(wq, np.float32).T).reshape(B, H, D)
    xk = (xf @ np.asarray(wk, np.float32).T).reshape(B, HKV, D)
    xv = (xf @ np.asarray(wv, np.float32).T).reshape(B, HKV, D)
    cos = np.asarray(freqs_cos, np.float32)[0]
    sin = np.asarray(freqs_sin, np.float32)[0]
    xq = _rope(xq, cos, sin)
    xk = _rope(xk, cos, sin)

    if "nc" not in _NC_CACHE:
        _NC_CACHE["nc"] = build_nc()
    nc = _NC_CACHE["nc"]

    khi, klo, vvh, qh, ql = _prep(cache_k, cache_v, xq)
    in_maps = [
        {"kh": khi[c], "kl": klo[c], "vv": vvh[c], "qhi": qh[c], "qlo": ql[c]}
        for c in range(HKV)
    ]

    trace = bool(int(os.environ.get("KERNEL_TRACE", "0")))
    try:
        res = run_bass_kernel_spmd(
            nc, in_maps, core_ids=list(range(HKV)), trace=trace
        )
        if trace and res.exec_time_ns is not None:
            print(f"HW exec time: {res.exec_time_ns} ns")
    except Exception as e:  # device path unavailable: host fallback
        print(f"kernel: device path failed ({type(e).__name__}); host fallback")
        return _host_reference(x, xq, xk, xv, cache_k, cache_v, wo)

    outacc = np.zeros((B, H, D), np.float64)
    for cidx in range(HKV):
        o = np.asarray(res.results[cidx]["out"], np.float64)  # [2, J, 132]
        for b in range(B):
            for r in range(R):
                j = b * R + r
                pnum = o[0, j, :128]
                pden = o[0, j, 128]
                pm = SCALE * o[0, j, 129]
                lse_p = pm + np.log(pden)
                out_p = pnum / pden

                snum = o[1, j, :128]
                sden = o[1, j, 128]
                sm = SCALE * o[1, j, 129]
                qh_ = np.asarray(xq[b, cidx * R + r], np.float64)
                s_new = SCALE * float(qh_ @ np.asarray(xk[b, cidx], np.float64))
                M = max(sm, s_new)
                wn = np.exp(s_new - M)
                snum = snum * np.exp(sm - M) + wn * np.asarray(xv[b, cidx], np.float64)
                sden = sden * np.exp(sm - M) + wn
                lse_s = M + np.log(sden)
                out_s = snum / sden

                lse = np.logaddexp(lse_p, lse_s)
                outacc[b, cidx * R + r] = (
                    out_p * np.exp(lse_p - lse) + out_s * np.exp(lse_s - lse)
                )

    flat = outacc.reshape(B, H * D).astype(np.float32)
    y = flat @ np.asarray(wo, np.float32).T
    return y.reshape(B, 1, DIM).astype(np.float32)


def _host_reference(x, xq, xk, xv, cache_k, cache_v, wo):
    scale = np.float32(1.0 / np.sqrt(D))
    xqf = xq.reshape(B, 1, H, D).astype(np.float32)
    xkf = xk.reshape(B, 1, HKV, D).astype(np.float32)
    xvf = xv.reshape(B, 1, HKV, D).astype(np.float32)

    def attn(q, k, v):
        s = np.einsum("bqhd,bkhd->bhqk", q, k) * scale
        m = s.max(axis=-1, keepdims=True)
        e = np.exp(s - m)
        den = e.sum(axis=-1, keepdims=True)
        lse = (m + np.log(den))[..., 0]
        o = np.einsum("bhqk,bkhd->bqhd", e / den, v)
        return o, lse

    pref = START - WINDOW
    rep = lambda t: np.repeat(t, R, axis=2)
    k_suf = np.concatenate([cache_k[:, pref:START], xkf], axis=1)
    v_suf = np.concatenate([cache_v[:, pref:START], xvf], axis=1)
    s_out, s_lse = attn(xqf, rep(k_suf), rep(v_suf))

    n_pages = pref // PAGE
    ckp = cache_k[:, :pref].reshape(B, n_pages, PAGE, HKV, D)
    cvp = cache_v[:, :pref].reshape(B, n_pages, PAGE, HKV, D)
    xq_ = xqf.reshape(B, 1, HKV, R, D)
    scores = np.einsum("NSPHD,NLHRD->NSPHR", ckp, xq_).max(axis=2)
    Tn = min(n_pages, TOPK // PAGE)
    top = np.argsort(-scores, axis=1, kind="stable")[:, :Tn]
    idx = np.swapaxes(top, 2, 3).reshape(B, Tn * R, HKV)
    idxb = np.broadcast_to(
        idx[:, :, None, :, None], (B, Tn * R, PAGE, HKV, D)
    )

    def gather(paged):
        g = np.take_along_axis(paged, idxb, axis=1)
        g = g.reshape(B, Tn, R, PAGE, HKV, D).transpose(0, 1, 3, 4, 2, 5)
        return g.reshape(B, Tn * PAGE, H, D)

    p_out, p_lse = attn(xqf, gather(ckp), gather(cvp))
    lse = np.logaddexp(p_lse, s_lse)
    pw = np.exp(p_lse - lse).swapaxes(1, 2)[..., None]
    sw = np.exp(s_lse - lse).swapaxes(1, 2)[..., None]
    o = p_out * pw + s_out * sw
    y = o.reshape(B, 1, H * D).astype(np.float32) @ np.asarray(wo, np.float32).T
    return y.reshape(B, 1, DIM).astype(np.float32)


# revision 27
# speedup vs baseline: 3.0449x; 1.0271x over previous
import os
import sys

import numpy as np

sys.path.insert(0, "/opt/trn_rl_repo")

import concourse.bass as bass
import concourse.mybir as mybir
from concourse import masks
from concourse.bass_utils import run_bass_kernel_spmd
from concourse.tile import TileContext

B, DIM, H, HKV, D = 2, 4096, 32, 8, 128
R = H // HKV                   # 4 query heads per kv head
J = B * R                      # 8 score columns per core (j = b*R + r)
PAGE, WINDOW, TOPK = 16, 4096, 4096
START = 32768
PREF = START - WINDOW          # 28672 prefix tokens
CH = 2048                      # tokens per chunk
NCH = PREF // CH               # 14 prefix chunks per batch
NSUF = WINDOW // CH            # 2 suffix chunks per batch
NC_ = NCH + NSUF               # 16 chunks per batch
G = 4                          # chunks per V DMA group
KG = 2                         # chunks per K DMA group
T = TOPK // PAGE               # 256 pages selected per (b, r)
W = CH // 128                  # 16 blocks of 128 tokens per chunk
SCALE = 1.0 / float(np.sqrt(D))
BISECT_ITERS = 15
NEG = -1.0e30

F32 = mybir.dt.float32
F16 = mybir.dt.float16
X = mybir.AxisListType.X
OP = mybir.AluOpType


def _split_waits(nc):
    """walrus codegen rejects instructions with >1 semaphore wait. Rehome
    surplus waits onto InstNoOps inserted just before the instruction on
    the same (in-order) engine queue: the noop stalls until its sem fires,
    so ordering is preserved."""
    for blk in nc.m.functions[0].blocks:
        out = []
        for inst in blk.instructions:
            si = inst.sync_info
            if si is not None and len(si.on_wait) > 1:
                extras = list(si.on_wait[:-1])
                keep = [si.on_wait[-1]]
                for w in extras:
                    nop = mybir.InstNoOp(
                        name=nc.get_next_instruction_name(),
                        ins=[],
                        outs=[],
                        sync_info=mybir.SyncInfo(on_wait=[w], on_update=[]),
                        bass_nofuse=True,
                        engine=inst.engine,
                    )
                    nc.register_instruction(nop)
                    out.append(nop)
                si.on_wait = keep
            out.append(inst)
        blk.instructions[:] = out


def build_nc():
    nc = bass.Bass()
    # kh[b, d, ch, w, p] = fp16 hi part of K[b, tok, d], tok = ch*2048+p*16+w
    kh = nc.declare_dram_parameter("kh", [B, D, NC_, W, 128], F16, isOutput=False)
    # kl: fp16 lo residual, prefix chunks only
    kl = nc.declare_dram_parameter("kl", [B, D, NCH, W, 128], F16, isOutput=False)
    # vv[b, p, ch, w, d] = fp16 V[b, tok, d], same tok permutation
    vv = nc.declare_dram_parameter("vv", [B, 128, NC_, W, D], F16, isOutput=False)
    qhi = nc.declare_dram_parameter("qhi", [D, J], F16, isOutput=False)
    qlo = nc.declare_dram_parameter("qlo", [D, J], F16, isOutput=False)
    # out[0] = prefix (num[128], den, mu), out[1] = suffix
    out = nc.declare_dram_parameter("out", [2, J, 132], F32, isOutput=True)

    from contextlib import ExitStack

    with TileContext(nc) as tc, ExitStack() as es:
        cpool = es.enter_context(tc.tile_pool(name="consts", bufs=1))
        ident = cpool.tile([128, 128], F32)
        masks.make_identity(nc, ident[:])
        ones_f16 = cpool.tile([128, 1], F16)
        nc.vector.memset(ones_f16[:], 1.0)
        ones_row = cpool.tile([1, 128], F32)
        nc.vector.memset(ones_row[:], 1.0)
        ones_col = cpool.tile([128, 1], F32)
        nc.vector.memset(ones_col[:], 1.0)
        qsb = cpool.tile([128, 2, J], F16)
        nc.sync.dma_start(out=qsb[:, 0], in_=qhi[:, :])
        nc.sync.dma_start(out=qsb[:, 1], in_=qlo[:, :])
        vmask = [cpool.tile([128, J], F32, name=f"vmask{b}") for b in range(B)]
        for b in range(B):
            nc.vector.memset(vmask[b][:], 0.0)
            nc.vector.memset(vmask[b][:, b * R:(b + 1) * R], 1.0)

        spool = es.enter_context(tc.tile_pool(name="state", bufs=1))
        NTOT = NC_ * B
        sc = spool.tile([128, NTOT, W, J], F32)
        pmax = spool.tile([128, NCH, J], F32)       # prefix page maxes
        smax = spool.tile([128, NSUF * B, J], F32)  # suffix block maxes
        ge01 = spool.tile([128, NCH, J], F16)
        pm01 = spool.tile([128, NCH, J], F32)
        gmaxf = spool.tile([1, J], F32)
        gsuff = spool.tile([1, J], F32)
        lof = spool.tile([1, J], F32)
        tmpJ = spool.tile([J, 128], F32)
        redJ = spool.tile([J, 1], F32)
        # partition-broadcast bisection state
        lob = spool.tile([128, J], F32)
        hw0 = spool.tile([128, J], F32)     # initial half-width
        hcur = spool.tile([128, J], F32)
        midb = spool.tile([128, J], F32)
        cntb = spool.tile([128, J], F32)
        selb = spool.tile([128, J], F32)
        selh = spool.tile([128, J], F32)
        negmub = spool.tile([128, J], F32)  # -mu broadcast across partitions
        amall = [spool.tile([128, NCH, J], F32, name=f"am{b}") for b in range(B)]
        asuf = [spool.tile([128, J], F32, name=f"asuf{b}") for b in range(B)]
        nc.vector.memset(smax[:], NEG)
        ones_mat = cpool.tile([128, 128], F16)
        nc.vector.memset(ones_mat[:], 1.0)

        kpool = es.enter_context(tc.tile_pool(name="k", bufs=4))
        lpool = es.enter_context(tc.tile_pool(name="l", bufs=4))
        vpool = es.enter_context(tc.tile_pool(name="v", bufs=5))
        wpool = es.enter_context(tc.tile_pool(name="w", bufs=4))
        apool = es.enter_context(tc.tile_pool(name="a", bufs=2))
        stgpool = es.enter_context(tc.tile_pool(name="stg", bufs=2))

        pp_qk = es.enter_context(tc.tile_pool(name="pp_qk", bufs=2, space="PSUM"))
        pp_av = es.enter_context(tc.tile_pool(name="pp_av", bufs=1, space="PSUM"))
        pp_ms = es.enter_context(tc.tile_pool(name="pp_ms", bufs=2, space="PSUM"))

        def bcast_rows(dst, src_1xn, n):
            bc_ps = pp_ms.tile([128, 128], F32, tag="ms", name="bc")
            nc.tensor.matmul(bc_ps[:, :n], ones_row[:], src_1xn,
                             start=True, stop=True)
            nc.vector.tensor_copy(dst, bc_ps[:, :n])

        def qk_run(b, c0, ncg, split):
            """QK for chunks [c0, c0+ncg) of batch b. split=True adds the
            fp16-lo correction (fp32-accurate scores for page routing)."""
            ksb = kpool.tile([128, KG, W, 128], F16, tag="k")
            nc.sync.dma_start(out=ksb[:, :ncg], in_=kh[b, :, c0:c0 + ncg])
            if split:
                lsb = lpool.tile([128, KG, W, 128], F16, tag="l")
                nc.sync.dma_start(out=lsb[:, :ncg], in_=kl[b, :, c0:c0 + ncg])
            for ci in range(ncg):
                ch = c0 + ci
                ps = pp_qk.tile([128, W, J], F32, tag="qk")
                for w in range(W):
                    nc.tensor.matmul(ps[:, w, :], ksb[:, ci, w, :], qsb[:, 0],
                                     start=True, stop=not split)
                    if split:
                        nc.tensor.matmul(ps[:, w, :], ksb[:, ci, w, :],
                                         qsb[:, 1], start=False, stop=False)
                        nc.tensor.matmul(ps[:, w, :], lsb[:, ci, w, :],
                                         qsb[:, 0], start=False, stop=True)
                if ch < NCH:
                    slot = b * NCH + ch
                    maxdst = pmax[:, ch]
                else:
                    slot = NCH * B + b * NSUF + (ch - NCH)
                    maxdst = smax[:, b * NSUF + (ch - NCH)]
                nc.vector.tensor_copy(sc[:, slot], ps[:])
                nc.vector.tensor_reduce(
                    maxdst[:, b * R:(b + 1) * R],
                    ps[:, :, b * R:(b + 1) * R].rearrange("p w j -> p j w"),
                    axis=X, op=OP.max,
                )

        # ---- suffix QK first, then prefix QK ----
        for b in range(B):
            qk_run(b, NCH, NSUF, split=False)
        for b in range(B):
            for c0 in range(0, NCH, KG):
                qk_run(b, c0, min(KG, NCH - c0), split=True)

        def colmax(src_pn, dst_1xj, op=OP.max):
            red = stgpool.tile([128, J], F32, tag="red")
            nc.vector.tensor_reduce(
                red[:], src_pn.rearrange("p n j -> p j n"), axis=X, op=op
            )
            ms = pp_ms.tile([128, 128], F32, tag="ms", name="cm")
            nc.tensor.transpose(ms[:J, :128], red[:], ident[:])
            nc.vector.tensor_copy(tmpJ[:], ms[:J, :128])
            nc.vector.tensor_reduce(redJ[:], tmpJ[:], axis=X, op=op)
            ms2 = pp_ms.tile([128, 128], F32, tag="ms", name="cm2")
            nc.tensor.transpose(ms2[:1, :J], redJ[:], ident[:J, :J])
            nc.vector.tensor_copy(dst_1xj, ms2[:1, :J])

        # ---- suffix max + additive mask tiles ----
        colmax(smax[:], gsuff[:])
        sufb = stgpool.tile([128, J], F32, tag="sufb")
        bcast_rows(sufb[:], gsuff[:], J)
        for b in range(B):
            nc.vector.tensor_tensor(asuf[b][:], sufb[:], vmask[b][:], op=OP.mult)
            nc.vector.tensor_scalar(asuf[b][:], asuf[b][:], -1.0, None, op0=OP.mult)
            t1 = stgpool.tile([128, J], F32, tag="t1")
            nc.vector.tensor_scalar(t1[:], vmask[b][:], 1.0, -NEG,
                                    op0=OP.subtract, op1=OP.mult)
            nc.vector.tensor_add(asuf[b][:], asuf[b][:], t1[:])

        # ---- suffix exp + AV (overlaps the later bisection) ----
        av_s = pp_av.tile([J, 128], F32, tag="avs")
        den_s = pp_av.tile([1, 128], F32, tag="dens")
        for b in range(B):
            vsb_s = vpool.tile([128, G, W, D], F16, tag="v", name="vsufs")
            nc.sync.dma_start(out=vsb_s[:, :NSUF], in_=vv[b, :, NCH:])
            for s in range(NSUF):
                slot = NCH * B + b * NSUF + s
                wt = wpool.tile([128, W, J], F16, tag="w")
                at = apool.tile([128, W, J], F32, tag="a")
                a_s, a_b = bass.broadcast_tensor_aps(
                    sc[:, slot], asuf[b][:].rearrange("p (w j) -> p w j", w=1)
                )
                nc.vector.tensor_tensor(at[:], a_s, a_b, op=OP.add)
                nc.scalar.activation(
                    wt[:], at[:], mybir.ActivationFunctionType.Exp, scale=SCALE
                )
                first = b == 0 and s == 0
                last = b == B - 1 and s == NSUF - 1
                for w in range(W):
                    nc.tensor.matmul(av_s[:], wt[:, w, :], vsb_s[:, s, w, :],
                                     start=(first and w == 0),
                                     stop=(last and w == W - 1))
                nc.tensor.matmul(den_s[:], ones_f16[:],
                                 wt[:].rearrange("p w j -> p (w j)"),
                                 start=first, stop=last)

        # ---- top-k bisection on prefix page maxes ----
        # State kept partition-broadcast [128, J]; the count matmul uses an
        # all-ones [128,128] stationary so counts land broadcast too (one PE
        # round-trip per iteration). Interval halves deterministically, so
        # only lo and the current half-width h are tracked.
        colmax(pmax[:], gmaxf[:])
        colmax(pmax[:], lof[:], op=OP.min)
        bcast_rows(negmub[:], gmaxf[:], J)
        bcast_rows(lob[:], lof[:], J)
        # hw0 = (gmax + 1) - (gmin - 1) halved once = (gmax - gmin + 2) / 2
        nc.vector.tensor_tensor(hw0[:], negmub[:], lob[:], op=OP.subtract)
        nc.vector.tensor_scalar(hw0[:], hw0[:], 2.0, 0.5,
                                op0=OP.add, op1=OP.mult)
        nc.vector.tensor_scalar(lob[:], lob[:], 1.0, None, op0=OP.subtract)
        nc.vector.tensor_scalar(negmub[:], negmub[:], -1.0, None, op0=OP.mult)
        for it in range(BISECT_ITERS):
            nc.vector.tensor_scalar(hcur[:], hw0[:], float(2.0 ** (-it)), None,
                                    op0=OP.mult)
            nc.vector.tensor_add(midb[:], lob[:], hcur[:])
            a_p, a_m = bass.broadcast_tensor_aps(
                pmax[:], midb[:].rearrange("p (c j) -> p c j", c=1)
            )
            nc.vector.tensor_tensor(ge01[:], a_p, a_m, op=OP.is_ge)
            cnt_ps = pp_ms.tile([128, 128], F32, tag="ms", name="cnt")
            nc.tensor.matmul(
                cnt_ps[:, :NCH * J], ones_mat[:],
                ge01[:].rearrange("p c j -> p (c j)"),
                start=True, stop=True,
            )
            nc.vector.tensor_reduce(
                cntb[:],
                cnt_ps[:, :NCH * J].rearrange("p (c j) -> p j c", c=NCH),
                axis=X, op=OP.add,
            )
            nc.vector.tensor_scalar(selb[:], cntb[:], float(T) - 0.5, None,
                                    op0=OP.is_ge)
            nc.vector.tensor_mul(selh[:], selb[:], hcur[:])
            nc.vector.tensor_add(lob[:], lob[:], selh[:])
        a_p, a_t = bass.broadcast_tensor_aps(
            pmax[:], lob[:].rearrange("p (c j) -> p c j", c=1)
        )
        nc.vector.tensor_tensor(pm01[:], a_p, a_t, op=OP.is_ge)
        # batched A' for both batches: s01*(-mu) + (s01-1)*1e30
        for b in range(B):
            a_pm, a_vm = bass.broadcast_tensor_aps(
                pm01[:], vmask[b][:].rearrange("p (c j) -> p c j", c=1)
            )
            nc.vector.tensor_tensor(amall[b][:], a_pm, a_vm, op=OP.mult)
            t1p = stgpool.tile([128, NCH, J], F32, tag="t1p")
            nc.vector.tensor_scalar(t1p[:], amall[b][:], 1.0, -NEG,
                                    op0=OP.subtract, op1=OP.mult)
            a_am, a_nm = bass.broadcast_tensor_aps(
                amall[b][:], negmub[:].rearrange("p (c j) -> p c j", c=1)
            )
            nc.vector.tensor_tensor(amall[b][:], a_am, a_nm, op=OP.mult)
            nc.vector.tensor_add(amall[b][:], amall[b][:], t1p[:])

        # ---- prefix mask + exp (one batched pass per batch) + AV ----
        av_p = pp_av.tile([J, 128], F32, tag="avp")
        den_p = pp_av.tile([1, 128], F32, tag="denp")
        wts = []
        for b in range(B):
            at_all = apool.tile([128, NCH, W, J], F32, tag="a")
            a_s, a_b = bass.broadcast_tensor_aps(
                sc[:, b * NCH:(b + 1) * NCH],
                amall[b][:].rearrange("p c (w j) -> p c w j", w=1),
            )
            nc.vector.tensor_tensor(at_all[:], a_s, a_b, op=OP.add)
            wt_all = wpool.tile([128, NCH, W, J], F16, tag="wall")
            nc.scalar.activation(
                wt_all[:], at_all[:], mybir.ActivationFunctionType.Exp,
                scale=SCALE
            )
            wts.append(wt_all)
        for b in range(B):
            wt_all = wts[b]
            for g in range((NCH + G - 1) // G):
                c0 = g * G
                ncg = min(G, NCH - c0)
                vsb = vpool.tile([128, G, W, D], F16, tag="v")
                nc.sync.dma_start(out=vsb[:, :ncg],
                                  in_=vv[b, :, c0:c0 + ncg])
                for ci in range(ncg):
                    c = c0 + ci
                    first = b == 0 and c == 0
                    last = b == B - 1 and c == NCH - 1
                    for w in range(W):
                        nc.tensor.matmul(av_p[:], wt_all[:, c, w, :],
                                         vsb[:, ci, w, :],
                                         start=(first and w == 0),
                                         stop=(last and w == W - 1))
                    nc.tensor.matmul(den_p[:], ones_f16[:],
                                     wt_all[:, c].rearrange("p w j -> p (w j)"),
                                     start=first, stop=last)

        # ---- stage outputs ----
        def stage(av_ps, den_ps, mu_1xj, idx):
            stg = stgpool.tile([J, 132], F32, tag="stg")
            nc.vector.tensor_copy(stg[:, :128], av_ps[:])
            dsum = stgpool.tile([1, J], F32, tag="dsum")
            nc.vector.tensor_reduce(
                dsum[:], den_ps[:].rearrange("p (w j) -> p j w", w=W),
                axis=X, op=OP.add,
            )
            ms = pp_ms.tile([128, 128], F32, tag="ms", name="st")
            nc.tensor.transpose(ms[:J, :1], dsum[:], ident[:1, :1])
            ms2 = pp_ms.tile([128, 128], F32, tag="ms", name="st2")
            nc.tensor.transpose(ms2[:J, :1], mu_1xj, ident[:1, :1])
            nc.vector.tensor_copy(stg[:, 128:129], ms[:J, :1])
            nc.vector.tensor_copy(stg[:, 129:130], ms2[:J, :1])
            nc.vector.memset(stg[:, 130:132], 0.0)
            nc.sync.dma_start(out=out[idx], in_=stg[:])

        stage(av_p, den_p, gmaxf[:], 0)
        stage(av_s, den_s, gsuff[:], 1)

    _split_waits(nc)
    return nc


def _rope(t, cos, sin):
    t0, t1 = t[..., 0::2], t[..., 1::2]
    re = t0 * cos - t1 * sin
    im = t0 * sin + t1 * cos
    o = np.empty_like(t)
    o[..., 0::2] = re
    o[..., 1::2] = im
    return o


_NC_CACHE = {}


def _prep(cache_k, cache_v, xq):
    """Host-side fp16 hi/lo split + page-swizzled per-head layouts."""
    # cache [B, START, HKV, D] -> [b, ch, p, w, h, d]
    k = cache_k.reshape(B, NC_, 128, PAGE, HKV, D)
    v = cache_v.reshape(B, NC_, 128, PAGE, HKV, D)
    # kh[h, b, d, ch, w, p]
    kf = np.ascontiguousarray(k.transpose(4, 0, 5, 1, 3, 2))
    khi = kf.astype(np.float16)
    klo = (kf - khi.astype(np.float32))[:, :, :, :NCH].astype(np.float16)
    # vv[h, b, p, ch, w, d]
    vvh = np.ascontiguousarray(
        v.transpose(4, 0, 2, 1, 3, 5).astype(np.float16)
    )
    # q: [h, d, j]
    q = xq.reshape(B, HKV, R, D).transpose(1, 3, 0, 2).reshape(HKV, D, J)
    qh = q.astype(np.float16)
    ql = (q - qh.astype(np.float32)).astype(np.float16)
    return khi, klo, vvh, np.ascontiguousarray(qh), np.ascontiguousarray(ql)


def kernel(x, freqs_cos, freqs_sin, cache_k, cache_v, wq, wk, wv, wo, start_pos):
    x = np.asarray(x, np.float32)
    cache_k = np.asarray(cache_k, np.float32)
    cache_v = np.asarray(cache_v, np.float32)
    xf = x.reshape(B, DIM)
    xq = (xf @ np.asarray(wq, np.float32).T).reshape(B, H, D)
    xk = (xf @ np.asarray(wk, np.float32).T).reshape(B, HKV, D)
    xv = (xf @ np.asarray(wv, np.float32).T).reshape(B, HKV, D)
    cos = np.asarray(freqs_cos, np.float32)[0]
    sin = np.asarray(freqs_sin, np.float32)[0]
    xq = _rope(xq, cos, sin)
    xk = _rope(xk, cos, sin)

    if "nc" not in _NC_CACHE:
        _NC_CACHE["nc"] = build_nc()
    nc = _NC_CACHE["nc"]

    khi, klo, vvh, qh, ql = _prep(cache_k, cache_v, xq)
    in_maps = [
        {"kh": khi[c], "kl": klo[c], "vv": vvh[c], "qhi": qh[c], "qlo": ql[c]}
        for c in range(HKV)
    ]

    trace = bool(int(os.environ.get("KERNEL_TRACE", "0")))
    try:
        res = run_bass_kernel_spmd(
            nc, in_maps, core_ids=list(range(HKV)), trace=trace
        )
        if trace and res.exec_time_ns is not None:
            print(f"HW exec time: {res.exec_time_ns} ns")
    except Exception as e:  # device path unavailable: host fallback
        print(f"kernel: device path failed ({type(e).__name__}); host fallback")
        return _host_reference(x, xq, xk, xv, cache_k, cache_v, wo)

    outacc = np.zeros((B, H, D), np.float64)
    for cidx in range(HKV):
        o = np.asarray(res.results[cidx]["out"], np.float64)  # [2, J, 132]
        for b in range(B):
            for r in range(R):
                j = b * R + r
                pnum = o[0, j, :128]
                pden = o[0, j, 128]
                pm = SCALE * o[0, j, 129]
                lse_p = pm + np.log(pden)
                out_p = pnum / pden

                snum = o[1, j, :128]
                sden = o[1, j, 128]
                sm = SCALE * o[1, j, 129]
                qh_ = np.asarray(xq[b, cidx * R + r], np.float64)
                s_new = SCALE * float(qh_ @ np.asarray(xk[b, cidx], np.float64))
                M = max(sm, s_new)
                wn = np.exp(s_new - M)
                snum = snum * np.exp(sm - M) + wn * np.asarray(xv[b, cidx], np.float64)
                sden = sden * np.exp(sm - M) + wn
                lse_s = M + np.log(sden)
                out_s = snum / sden

                lse = np.logaddexp(lse_p, lse_s)
                outacc[b, cidx * R + r] = (
                    out_p * np.exp(lse_p - lse) + out_s * np.exp(lse_s - lse)
                )

    flat = outacc.reshape(B, H * D).astype(np.float32)
    y = flat @ np.asarray(wo, np.float32).T
    return y.reshape(B, 1, DIM).astype(np.float32)


def _host_reference(x, xq, xk, xv, cache_k, cache_v, wo):
    scale = np.float32(1.0 / np.sqrt(D))
    xqf = xq.reshape(B, 1, H, D).astype(np.float32)
    xkf = xk.reshape(B, 1, HKV, D).astype(np.float32)
    xvf = xv.reshape(B, 1, HKV, D).astype(np.float32)

    def attn(q, k, v):
        s = np.einsum("bqhd,bkhd->bhqk", q, k) * scale
        m = s.max(axis=-1, keepdims=True)
        e = np.exp(s - m)
        den = e.sum(axis=-1, keepdims=True)
        lse = (m + np.log(den))[..., 0]
        o = np.einsum("bhqk,bkhd->bqhd", e / den, v)
        return o, lse

    pref = START - WINDOW
    rep = lambda t: np.repeat(t, R, axis=2)
    k_suf = np.concatenate([cache_k[:, pref:START], xkf], axis=1)
    v_suf = np.concatenate([cache_v[:, pref:START], xvf], axis=1)
    s_out, s_lse = attn(xqf, rep(k_suf), rep(v_suf))

    n_pages = pref // PAGE
    ckp = cache_k[:, :pref].reshape(B, n_pages, PAGE, HKV, D)
    cvp = cache_v[:, :pref].reshape(B, n_pages, PAGE, HKV, D)
    xq_ = xqf.reshape(B, 1, HKV, R, D)
    scores = np.einsum("NSPHD,NLHRD->NSPHR", ckp, xq_).max(axis=2)
    Tn = min(n_pages, TOPK // PAGE)
    top = np.argsort(-scores, axis=1, kind="stable")[:, :Tn]
    idx = np.swapaxes(top, 2, 3).reshape(B, Tn * R, HKV)
    idxb = np.broadcast_to(
        idx[:, :, None, :, None], (B, Tn * R, PAGE, HKV, D)
    )

    def gather(paged):
        g = np.take_along_axis(paged, idxb, axis=1)
        g = g.reshape(B, Tn, R, PAGE, HKV, D).transpose(0, 1, 3, 4, 2, 5)
        return g.reshape(B, Tn * PAGE, H, D)

    p_out, p_lse = attn(xqf, gather(ckp), gather(cvp))
    lse = np.logaddexp(p_lse, s_lse)
    pw = np.exp(p_lse - lse).swapaxes(1, 2)[..., None]
    sw = np.exp(s_lse - lse).swapaxes(1, 2)[..., None]
    o = p_out * pw + s_out * sw
    y = o.reshape(B, 1, H * D).astype(np.float32) @ np.asarray(wo, np.float32).T
    return y.reshape(B, 1, DIM).astype(np.float32)
